# revision 5
# baseline (speedup 1.0000x reference)
"""LoRA row-parallel linear on 8 TRN2 NeuronCores.

Problem: y = x @ W^T + delta, where per-token LoRA delta[t] = B[s] @ (A[s] @ x[t]),
s = token_to_slot[t] (8 adapters, rank 16, scaling baked into B).

Strategy: token data-parallel across the 8 cores (T=8192 -> 1024 tokens/core).
No collectives needed; each core computes its token block fully:
  u^T  = A_all @ x_shard^T          (128 x T_SH; A_all = all 8 adapters stacked)
  uM^T = u^T * mask^T               (one-hot select of each token's adapter)
  y    = x @ W^T + uM @ B_all       (PSUM accumulation: 32 k-tiles of W + 1 of B)
All matmuls run as float32r (TF32-like, FP22) at full PE rate with fp32 accumulate.

Schedule (per core): the first output-column block (ob0) runs its 32-k-tile
d-loop FIRST, so the PE has work while the 16 MB x^T shard streams in; the
u-pass (which needs the whole shard) runs after it, and ob0's LoRA delta is
applied as a separate accumulation + DVE add. Remaining obs fuse the delta as
a 33rd accumulation step.

Host prep: transposes x/W/A to put the contraction dim on partitions, builds
the one-hot mask from token_to_slot. Device does all the FLOPs.
"""

import numpy as np

from concourse import bacc, tile, mybir
from concourse.bass_utils import run_bass_kernel_spmd
import concourse.bass_utils as _bu

# Disable S3 artifact upload in the trace path (no credentials in this container).
_bu.upload_artifacts = lambda tmpdir: "local://" + tmpdir

N_CORES = 8
T = 8192
D_IN = 4096
D_OUT = 4096
L = 8          # max adapters
R = 16         # max rank
LR = L * R     # 128 = stacked adapter dim
T_SH = T // N_CORES          # 1024 tokens per core
KT = D_IN // 128             # 32 contraction tiles
OB = D_OUT // 512            # 8 output-column blocks
TB = T_SH // 128             # 8 token blocks per core
UB = T_SH // 512             # 2 token superblocks for the u-pass

F32 = mybir.dt.float32
F32R = mybir.dt.float32r

_CACHED_NC = None


def _build():
    nc = bacc.Bacc("TRN2", target_bir_lowering=False, debug=False)

    xT_d = nc.dram_tensor("xT", [D_IN, T_SH], F32, kind="ExternalInput")
    wT_d = nc.dram_tensor("wT", [D_IN, D_OUT], F32, kind="ExternalInput")
    aT_d = nc.dram_tensor("aT", [D_IN, LR], F32, kind="ExternalInput")
    bC_d = nc.dram_tensor("bC", [LR, D_OUT], F32, kind="ExternalInput")
    mT_d = nc.dram_tensor("maskT", [LR, T_SH], F32, kind="ExternalInput")
    y_d = nc.dram_tensor("y", [T_SH, D_OUT], F32, kind="ExternalOutput")

    with tile.TileContext(nc) as tc:
        with (
            tc.tile_pool(name="resident", bufs=1) as rpool,
            tc.tile_pool(name="wstream", bufs=4) as wpool,
            tc.tile_pool(name="yout", bufs=6) as ypool,
            tc.tile_pool(name="psum", bufs=8, space="PSUM") as psum,
        ):
            # --- resident loads; xts interleaved with ob0's w tiles so the
            # --- ob0 d-loop can start as soon as the first k-tile lands.
            xts = []
            wts0 = []
            for d in range(KT):
                xt = rpool.tile([128, T_SH], F32R, tag=f"xt{d}")
                nc.sync.dma_start(xt[:], xT_d[d * 128:(d + 1) * 128, :].bitcast(F32R))
                xts.append(xt)
                wt = wpool.tile([128, 512], F32R, tag="wt", name=f"wt0_{d}")
                nc.sync.dma_start(wt[:], wT_d[d * 128:(d + 1) * 128, 0:512].bitcast(F32R))
                wts0.append(wt)
            ats = []
            for d in range(KT):
                at = rpool.tile([128, LR], F32R, tag=f"at{d}")
                nc.sync.dma_start(at[:], aT_d[d * 128:(d + 1) * 128, :].bitcast(F32R))
                ats.append(at)
            bc = rpool.tile([LR, D_OUT], F32R, tag="bc")
            nc.sync.dma_start(bc[:], bC_d[:].bitcast(F32R))
            mask = rpool.tile([LR, T_SH], F32, tag="mask")
            nc.sync.dma_start(mask[:], mT_d[:])
            uTms = [rpool.tile([LR, 512], F32R, tag=f"uTm{ub}", name=f"uTm{ub}")
                    for ub in range(UB)]

            # --- phase 1: ob0 d-loop (base matmul only, no delta) --------------
            pys0 = [psum.tile([128, 512], F32, tag="acc", name=f"py0_{t}")
                    for t in range(TB)]
            for d in range(KT):
                for t in range(TB):
                    nc.tensor.matmul(
                        pys0[t][:], xts[d][:, t * 128:(t + 1) * 128], wts0[d][:],
                        start=(d == 0), stop=(d == KT - 1), skip_group_check=True,
                    )
            yo0s = []
            for t in range(TB):
                yo0 = rpool.tile([128, 512], F32, tag=f"yo0_{t}")
                if t % 2 == 0:
                    nc.vector.tensor_copy(yo0[:], pys0[t][:])
                else:
                    nc.scalar.copy(yo0[:], pys0[t][:])
                yo0s.append(yo0)

            # --- phase 2: u-pass (needs all xts, which have landed by now) -----
            for ub in range(UB):
                pu = psum.tile([128, 512], F32, tag="acc", name=f"pu{ub}")
                sl = slice(ub * 512, (ub + 1) * 512)
                for d in range(KT):
                    nc.tensor.matmul(
                        pu[:], ats[d][:], xts[d][:, sl],
                        start=(d == 0), stop=(d == KT - 1), skip_group_check=True,
                    )
                nc.vector.tensor_mul(uTms[ub][:], pu[:], mask[:, sl])

            # --- phase 3: ob0 delta + writeback --------------------------------
            for t in range(TB):
                pd = psum.tile([128, 512], F32, tag="acc", name=f"pd{t}")
                nc.tensor.matmul(
                    pd[:], uTms[t // 4][:, (t % 4) * 128:(t % 4 + 1) * 128], bc[:, 0:512],
                    start=True, stop=True, skip_group_check=True,
                )
                yo = ypool.tile([128, 512], F32, tag="yo", name=f"yod{t}")
                nc.vector.tensor_add(yo[:], yo0s[t][:], pd[:])
                nc.sync.dma_start(y_d[t * 128:(t + 1) * 128, 0:512], yo[:])

            # --- phase 4: ob1..7 with fused delta ------------------------------
            for ob in range(1, OB):
                osl = slice(ob * 512, (ob + 1) * 512)
                pys = [psum.tile([128, 512], F32, tag="acc", name=f"py{ob}_{t}")
                       for t in range(TB)]
                for d in range(KT):
                    wt = wpool.tile([128, 512], F32R, tag="wt", name=f"wt{ob}_{d}")
                    nc.sync.dma_start(wt[:], wT_d[d * 128:(d + 1) * 128, osl].bitcast(F32R))
                    for t in range(TB):
                        nc.tensor.matmul(
                            pys[t][:], xts[d][:, t * 128:(t + 1) * 128], wt[:],
                            start=(d == 0), stop=False, skip_group_check=True,
                        )
                for t in range(TB):
                    nc.tensor.matmul(
                        pys[t][:], uTms[t // 4][:, (t % 4) * 128:(t % 4 + 1) * 128],
                        bc[:, osl],
                        start=False, stop=True, skip_group_check=True,
                    )
                    yo = ypool.tile([128, 512], F32, tag="yo", name=f"yo{ob}_{t}")
                    if t % 2 == 0:
                        nc.vector.tensor_copy(yo[:], pys[t][:])
                    else:
                        nc.scalar.copy(yo[:], pys[t][:])
                    nc.sync.dma_start(y_d[t * 128:(t + 1) * 128, osl], yo[:])

    nc.compile()
    return nc


def _get_nc():
    global _CACHED_NC
    if _CACHED_NC is None:
        _CACHED_NC = _build()
    return _CACHED_NC


def _prep_in_maps(x, weight, lora_A, lora_B, token_to_slot):
    x = np.asarray(x, dtype=np.float32)
    weight = np.asarray(weight, dtype=np.float32)
    lora_A = np.asarray(lora_A, dtype=np.float32)
    lora_B = np.asarray(lora_B, dtype=np.float32)
    slots = np.asarray(token_to_slot)

    wT = np.ascontiguousarray(weight.T)                                    # [D_IN, D_OUT]
    aT = np.ascontiguousarray(lora_A.transpose(2, 0, 1).reshape(D_IN, LR))  # [D_IN, L*R]
    bC = np.ascontiguousarray(lora_B.transpose(0, 2, 1).reshape(LR, D_OUT)) # [L*R, D_OUT]

    # One-hot mask over stacked adapter rows; out-of-range slots -> all-zero.
    maskT = np.zeros((LR, T), dtype=np.float32)
    for l in range(L):
        maskT[l * R:(l + 1) * R, :] = (slots == l).astype(np.float32)[None, :]

    in_maps = []
    for c in range(N_CORES):
        tsl = slice(c * T_SH, (c + 1) * T_SH)
        in_maps.append({
            "xT": np.ascontiguousarray(x[tsl, :].T),
            "wT": wT,
            "aT": aT,
            "bC": bC,
            "maskT": np.ascontiguousarray(maskT[:, tsl]),
        })
    return in_maps


def _run(inputs, trace=False, trace_cores=None):
    nc = _get_nc()
    in_maps = _prep_in_maps(**inputs)
    res = run_bass_kernel_spmd(
        nc, in_maps, core_ids=list(range(N_CORES)),
        trace=trace, trace_cores=trace_cores,
    )
    y = np.concatenate([res.results[c]["y"] for c in range(N_CORES)], axis=0)
    return y, res


def kernel(x, weight, lora_A, lora_B, token_to_slot):
    y, _ = _run(dict(x=x, weight=weight, lora_A=lora_A, lora_B=lora_B,
                     token_to_slot=token_to_slot))
    return y


# revision 6
# speedup vs baseline: 1.0508x; 1.0508x over previous
"""LoRA row-parallel linear on 8 TRN2 NeuronCores.

Problem: y = x @ W^T + delta, where per-token LoRA delta[t] = B[s] @ (A[s] @ x[t]),
s = token_to_slot[t] (8 adapters, rank 16, scaling baked into B).

Strategy: token data-parallel across the 8 cores (T=8192 -> 1024 tokens/core).
No collectives needed; each core computes its token block fully:
  u^T  = A_all @ x_shard^T          (128 x T_SH; A_all = all 8 adapters stacked)
  uM^T = u^T * mask^T               (one-hot select of each token's adapter)
  y    = x @ W^T + uM @ B_all       (PSUM accumulation: 32 k-tiles of W + 1 of B)
All matmuls run as float32r (TF32-like, FP22) at full PE rate with fp32 accumulate.

Schedule (per core): the first output-column block (ob0) runs its 32-k-tile
d-loop FIRST, so the PE has work while the 16 MB x^T shard streams in; the
u-pass (which needs the whole shard) runs after it, and ob0's LoRA delta is
applied as a separate accumulation + DVE add. Remaining obs fuse the delta as
a 33rd accumulation step.

Host prep: transposes x/W/A to put the contraction dim on partitions, builds
the one-hot mask from token_to_slot. Device does all the FLOPs.
"""

import numpy as np

from concourse import bacc, tile, mybir
from concourse.bass_utils import run_bass_kernel_spmd
import concourse.bass_utils as _bu

# Disable S3 artifact upload in the trace path (no credentials in this container).
_bu.upload_artifacts = lambda tmpdir: "local://" + tmpdir

N_CORES = 8
T = 8192
D_IN = 4096
D_OUT = 4096
L = 8          # max adapters
R = 16         # max rank
LR = L * R     # 128 = stacked adapter dim
T_SH = T // N_CORES          # 1024 tokens per core
KT = D_IN // 128             # 32 contraction tiles
OB = D_OUT // 512            # 8 output-column blocks
TB = T_SH // 128             # 8 token blocks per core
UB = T_SH // 512             # 2 token superblocks for the u-pass

F32 = mybir.dt.float32
F32R = mybir.dt.float32r

_CACHED_NC = None


def _build():
    nc = bacc.Bacc("TRN2", target_bir_lowering=False, debug=False)

    xT_d = nc.dram_tensor("xT", [D_IN, T_SH], F32, kind="ExternalInput")
    wT_d = nc.dram_tensor("wT", [D_IN, D_OUT], F32, kind="ExternalInput")
    aT_d = nc.dram_tensor("aT", [D_IN, LR], F32, kind="ExternalInput")
    bC_d = nc.dram_tensor("bC", [LR, D_OUT], F32, kind="ExternalInput")
    mT_d = nc.dram_tensor("maskT", [LR, T_SH], F32, kind="ExternalInput")
    y_d = nc.dram_tensor("y", [T_SH, D_OUT], F32, kind="ExternalOutput")

    with tile.TileContext(nc) as tc:
        with (
            tc.tile_pool(name="resident", bufs=1) as rpool,
            tc.tile_pool(name="wstream", bufs=4) as wpool,
            tc.tile_pool(name="yout", bufs=6) as ypool,
            tc.tile_pool(name="psum", bufs=8, space="PSUM") as psum,
        ):
            # --- resident loads; xts interleaved with ob0's w tiles so the
            # --- ob0 d-loop can start as soon as the first k-tile lands.
            xts = []
            wts0 = []
            for d in range(KT):
                xt = rpool.tile([128, T_SH], F32R, tag=f"xt{d}")
                nc.sync.dma_start(xt[:], xT_d[d * 128:(d + 1) * 128, :].bitcast(F32R))
                xts.append(xt)
                wt = wpool.tile([128, 512], F32R, tag="wt", name=f"wt0_{d}")
                nc.sync.dma_start(wt[:], wT_d[d * 128:(d + 1) * 128, 0:512].bitcast(F32R))
                wts0.append(wt)
            ats = []
            for d in range(KT):
                at = rpool.tile([128, LR], F32R, tag=f"at{d}")
                nc.sync.dma_start(at[:], aT_d[d * 128:(d + 1) * 128, :].bitcast(F32R))
                ats.append(at)
            bc = rpool.tile([LR, D_OUT], F32R, tag="bc")
            nc.sync.dma_start(bc[:], bC_d[:].bitcast(F32R))
            mask = rpool.tile([LR, T_SH], F32, tag="mask")
            nc.sync.dma_start(mask[:], mT_d[:])
            uTms = [rpool.tile([LR, 512], F32R, tag=f"uTm{ub}", name=f"uTm{ub}")
                    for ub in range(UB)]

            # --- phase 1: ob0 d-loop (base matmul only, no delta) --------------
            pys0 = [psum.tile([128, 512], F32, tag="acc", name=f"py0_{t}")
                    for t in range(TB)]
            for d in range(KT):
                for t in range(TB):
                    nc.tensor.matmul(
                        pys0[t][:], xts[d][:, t * 128:(t + 1) * 128], wts0[d][:],
                        start=(d == 0), stop=(d == KT - 1), skip_group_check=True,
                    )
            yo0s = []
            for t in range(TB):
                yo0 = rpool.tile([128, 512], F32, tag=f"yo0_{t}")
                nc.vector.tensor_copy(yo0[:], pys0[t][:])
                yo0s.append(yo0)

            # --- phase 2: u-pass (needs all xts, which have landed by now) -----
            for ub in range(UB):
                pu = psum.tile([128, 512], F32, tag="acc", name=f"pu{ub}")
                sl = slice(ub * 512, (ub + 1) * 512)
                for d in range(KT):
                    nc.tensor.matmul(
                        pu[:], ats[d][:], xts[d][:, sl],
                        start=(d == 0), stop=(d == KT - 1), skip_group_check=True,
                    )
                nc.vector.tensor_mul(uTms[ub][:], pu[:], mask[:, sl])

            # --- phase 3: ob0 delta + writeback --------------------------------
            for t in range(TB):
                pd = psum.tile([128, 512], F32, tag="acc", name=f"pd{t}")
                nc.tensor.matmul(
                    pd[:], uTms[t // 4][:, (t % 4) * 128:(t % 4 + 1) * 128], bc[:, 0:512],
                    start=True, stop=True, skip_group_check=True,
                )
                yo = ypool.tile([128, 512], F32, tag="yo", name=f"yod{t}")
                nc.vector.tensor_add(yo[:], yo0s[t][:], pd[:])
                nc.sync.dma_start(y_d[t * 128:(t + 1) * 128, 0:512], yo[:])

            # --- phase 4: ob1..7 with fused delta ------------------------------
            for ob in range(1, OB):
                osl = slice(ob * 512, (ob + 1) * 512)
                pys = [psum.tile([128, 512], F32, tag="acc", name=f"py{ob}_{t}")
                       for t in range(TB)]
                for d in range(KT):
                    wt = wpool.tile([128, 512], F32R, tag="wt", name=f"wt{ob}_{d}")
                    nc.sync.dma_start(wt[:], wT_d[d * 128:(d + 1) * 128, osl].bitcast(F32R))
                    for t in range(TB):
                        nc.tensor.matmul(
                            pys[t][:], xts[d][:, t * 128:(t + 1) * 128], wt[:],
                            start=(d == 0), stop=False, skip_group_check=True,
                        )
                for t in range(TB):
                    nc.tensor.matmul(
                        pys[t][:], uTms[t // 4][:, (t % 4) * 128:(t % 4 + 1) * 128],
                        bc[:, osl],
                        start=False, stop=True, skip_group_check=True,
                    )
                    yo = ypool.tile([128, 512], F32, tag="yo", name=f"yo{ob}_{t}")
                    nc.vector.tensor_copy(yo[:], pys[t][:])
                    nc.sync.dma_start(y_d[t * 128:(t + 1) * 128, osl], yo[:])

    nc.compile()
    return nc


def _get_nc():
    global _CACHED_NC
    if _CACHED_NC is None:
        _CACHED_NC = _build()
    return _CACHED_NC


def _prep_in_maps(x, weight, lora_A, lora_B, token_to_slot):
    x = np.asarray(x, dtype=np.float32)
    weight = np.asarray(weight, dtype=np.float32)
    lora_A = np.asarray(lora_A, dtype=np.float32)
    lora_B = np.asarray(lora_B, dtype=np.float32)
    slots = np.asarray(token_to_slot)

    wT = np.ascontiguousarray(weight.T)                                    # [D_IN, D_OUT]
    aT = np.ascontiguousarray(lora_A.transpose(2, 0, 1).reshape(D_IN, LR))  # [D_IN, L*R]
    bC = np.ascontiguousarray(lora_B.transpose(0, 2, 1).reshape(LR, D_OUT)) # [L*R, D_OUT]

    # One-hot mask over stacked adapter rows; out-of-range slots -> all-zero.
    maskT = np.zeros((LR, T), dtype=np.float32)
    for l in range(L):
        maskT[l * R:(l + 1) * R, :] = (slots == l).astype(np.float32)[None, :]

    in_maps = []
    for c in range(N_CORES):
        tsl = slice(c * T_SH, (c + 1) * T_SH)
        in_maps.append({
            "xT": np.ascontiguousarray(x[tsl, :].T),
            "wT": wT,
            "aT": aT,
            "bC": bC,
            "maskT": np.ascontiguousarray(maskT[:, tsl]),
        })
    return in_maps


def _run(inputs, trace=False, trace_cores=None):
    nc = _get_nc()
    in_maps = _prep_in_maps(**inputs)
    res = run_bass_kernel_spmd(
        nc, in_maps, core_ids=list(range(N_CORES)),
        trace=trace, trace_cores=trace_cores,
    )
    y = np.concatenate([res.results[c]["y"] for c in range(N_CORES)], axis=0)
    return y, res


def kernel(x, weight, lora_A, lora_B, token_to_slot):
    y, _ = _run(dict(x=x, weight=weight, lora_A=lora_A, lora_B=lora_B,
                     token_to_slot=token_to_slot))
    return y
